# revision 1
# baseline (speedup 1.0000x reference)
"""Trainium2 Bass kernel for nn_KuramotoHyperUniversal.

Data-parallel over batch across 8 NeuronCores (64 rows/core); weights
replicated. The (B,D,D) pairwise term is computed via the identity
  sum_j sin(y_j - y_i) A[i,j] = cos(y_i)*(A@sin(y))_i - sin(y_i)*(A@cos(y))_i
so it becomes two [64,512]x[512,512] matmuls instead of a 64MB tensor.
The constant t-column of the MLP input is folded into the layer-0 bias.
Matmuls run as fp32r (single-pass fp32) with activations transposed
(feature-on-partition) as the stationary operand and weights streamed.
"""

import numpy as np
from contextlib import ExitStack

import concourse.bass as bass
import concourse.mybir as mybir
import concourse.tile as tile
from concourse.vector_clock import ScopedClock, VectorClock
from concourse.bass_utils import run_bass_kernel_spmd
from concourse.masks import make_identity

DIM = 512
BATCH = 512
NCORES = 8
BS = BATCH // NCORES  # 64
H = 2 + 3 * DIM  # 1538
IN_SZ = 1 + 3 * DIM  # 1537
F32 = mybir.dt.float32
F32R = mybir.dt.float32r
PI_HALF = float(np.pi / 2.0)


def _split_drain_and_barrier(self, tick_clock, wait_clock):
    # Walrus in this container rejects >2 sync waits on one CTRL (drain)
    # instruction; emit one single-wait NOP per outstanding proc instead.
    gc = tick_clock.global_clock
    ticks = list(gc)
    for p, t in enumerate(ticks):
        if t > 0:
            v = [0] * len(ticks)
            v[p] = t
            nop = self.nc.sync.nop(nofuse=True, hint=f"drain_wait_{p}")
            wait_clock.add_sem_waits(nop.ins, ScopedClock({None: VectorClock(v)}))
    self.nc.sync.drain()
    self.nc.all_engine_barrier()
    popped = self.nc._tile_sem_poison_stack.pop()
    assert popped is self._sem_poison
    self.nc.clear_and_free_semaphores(list(self.sems.allocated().values()))
    self.nc.all_engine_barrier()


tile.TileContext._drain_and_barrier = _split_drain_and_barrier


def _r(ap):
    return ap.bitcast(F32R)


_MAX_WAITS = 1


def _split_waits(nc, limit=_MAX_WAITS):
    """Walrus rejects instructions carrying more than `limit` sync waits;
    move the excess onto same-engine NOPs inserted just before."""
    import bass_rust

    n = 0
    for f in nc.m.functions:
        for bb in f.blocks:
            out = []
            for inst in bb.instructions:
                si = inst.sync_info
                if si is not None and si.on_wait and len(si.on_wait) > limit:
                    waits = list(si.on_wait)
                    extra, keep = waits[:-limit], waits[-limit:]
                    for i in range(0, len(extra), limit):
                        nop = mybir.InstNoOp(name=f"I-wsplit-{n}", engine=inst.engine)
                        n += 1
                        nop.sync_info = bass_rust.SyncInfo(
                            on_wait=extra[i : i + limit], on_update=[]
                        )
                        out.append(nop)
                    inst.sync_info = bass_rust.SyncInfo(
                        on_wait=keep, on_update=list(si.on_update)
                    )
                out.append(inst)
            bb.instructions = out


def _build(w_bufs=12, reps=1):
    nc = bass.Bass()
    AF = mybir.ActivationFunctionType

    t_p = nc.declare_dram_parameter("t", [1], F32, isOutput=False)
    y_p = nc.declare_dram_parameter("y", [BS, DIM + 1], F32, isOutput=False)
    fr_p = nc.declare_dram_parameter("freqs", [BS, DIM], F32, isOutput=False)
    A_p = nc.declare_dram_parameter("A", [DIM, DIM], F32, isOutput=False)
    W_p = [
        nc.declare_dram_parameter("W0", [IN_SZ, H], F32, isOutput=False),
        nc.declare_dram_parameter("W1", [H, H], F32, isOutput=False),
        nc.declare_dram_parameter("W2", [H, H], F32, isOutput=False),
        nc.declare_dram_parameter("W3", [H, DIM], F32, isOutput=False),
    ]
    b_p = [
        nc.declare_dram_parameter("b0", [H], F32, isOutput=False),
        nc.declare_dram_parameter("b1", [H], F32, isOutput=False),
        nc.declare_dram_parameter("b2", [H], F32, isOutput=False),
        nc.declare_dram_parameter("b3", [DIM], F32, isOutput=False),
    ]
    out_p = nc.declare_dram_parameter("out", [BS, DIM + 1], F32, isOutput=True)

    with ExitStack() as ctx:
        tc = ctx.enter_context(tile.TileContext(nc))
        const = ctx.enter_context(tc.tile_pool(name="const", bufs=1))
        io = ctx.enter_context(tc.tile_pool(name="io", bufs=1))
        xtp = ctx.enter_context(tc.tile_pool(name="xtp", bufs=1))
        atp = ctx.enter_context(tc.tile_pool(name="atp", bufs=1))
        htp = ctx.enter_context(tc.tile_pool(name="htp", bufs=2))
        wp = ctx.enter_context(tc.tile_pool(name="wp", bufs=w_bufs))
        ain = ctx.enter_context(tc.tile_pool(name="ain", bufs=2))
        ps = ctx.enter_context(tc.tile_pool(name="ps", bufs=1, space="PSUM"))
        pst = ctx.enter_context(tc.tile_pool(name="pst", bufs=2, space="PSUM"))

        id64 = const.tile([64, 64], F32, tag="id64")
        make_identity(nc, id64[:])
        id128 = const.tile([128, 128], F32, tag="id128")
        make_identity(nc, id128[:])
        ones_f = const.tile([1, 64], F32, tag="ones_f")
        nc.vector.memset(ones_f[:], 1.0)
        ones = const.tile([1, 64], F32R, tag="ones")
        nc.vector.tensor_copy(ones[:], ones_f[:])
        pih = const.tile([BS, 1], F32, tag="pih")
        nc.vector.memset(pih[:], PI_HALF)

        def _emit(rep):
            # ---- inputs ----
            yd = io.tile([BS, DIM], F32, tag="yd")
            nc.sync.dma_start(out=yd[:], in_=y_p[:, 0:DIM])
            fr = io.tile([BS, DIM], F32, tag="fr")
            nc.sync.dma_start(out=fr[:], in_=fr_p[:])
            t_sb = const.tile([1, 1], F32, tag="t")
            nc.sync.dma_start(out=t_sb[:], in_=t_p[None, :])

            # C = cos(yd) = sin(yd + pi/2), S = sin(yd)   [64, 512]
            C = io.tile([BS, DIM], F32, tag="C")
            nc.scalar.activation(C[:], yd[:], AF.Sin, bias=pih[:])
            S = io.tile([BS, DIM], F32, tag="S")
            nc.scalar.activation(S[:], yd[:], AF.Sin)

            # ---- transposed copies (feature-on-partition, [128, 64] tiles) ----
            def transpose4(src, pref):
                tiles = []
                for j in range(4):
                    p = pst.tile([128, 64], F32, tag="pstT")
                    nc.tensor.transpose(p[:], src[:, j * 128 : (j + 1) * 128], id64[:])
                    tt = xtp.tile([128, 64], F32R, tag=f"{pref}{j}")
                    nc.vector.tensor_copy(tt[:], p[:])
                    tiles.append(tt)
                return tiles

            xC = transpose4(C, "xC")
            xS = transpose4(S, "xS")
            xF = transpose4(fr, "xF")

            # ---- A transposed: AT_j [128, 512], partition = col index of A ----
            AT = [
                atp.tile([128, DIM], F32R, tag=f"AT{j}", name=f"AT{j}") for j in range(4)
            ]
            for i in range(4):
                arow = ain.tile([128, DIM], F32, tag="arow")
                nc.sync.dma_start(out=arow[:], in_=A_p[i * 128 : (i + 1) * 128, :])
                for j in range(4):
                    p = pst.tile([128, 128], F32, tag="pstA", bufs=1)
                    nc.tensor.transpose(p[:], arow[:, j * 128 : (j + 1) * 128], id128[:])
                    nc.vector.tensor_copy(AT[j][:, i * 128 : (i + 1) * 128], p[:])

            # ---- biases ----
            brow = []
            for l, (bp, n) in enumerate(zip(b_p, [H, H, H, DIM])):
                bt = const.tile([1, n], F32R, tag=f"b{l}", name=f"b{l}row")
                nc.sync.dma_start(out=bt[:], in_=bp[None, :].bitcast(F32R))
                brow.append(bt)
            # b0' = b0 + (t-1) * W0[1024, :]
            w0row = const.tile([1, H], F32, tag="w0row")
            nc.sync.dma_start(out=w0row[:], in_=W_p[0][1024:1025, :])
            tm1 = const.tile([1, 1], F32, tag="tm1")
            nc.vector.tensor_scalar_add(tm1[:], t_sb[:], -1.0)
            b0p = const.tile([1, H], F32R, tag="b0p")
            nc.vector.tensor_scalar_mul(b0p[:], w0row[:], tm1[:])
            nc.vector.tensor_add(b0p[:], b0p[:], brow[0][:])

            # ---- forcesum = C*(S@A^T) - S*(C@A^T) ----
            # AS[b,i] = sum_j S[b,j] A[i,j]:  lhsT=xS_j [128,64], rhs=AT_j [128,512]
            fs = io.tile([BS, DIM], F32, tag="fs")
            for name, xt in (("AS", xS), ("AC", xC)):
                ptr = pst.tile([BS, DIM], F32, tag="pstrig", bufs=1)
                for j in range(4):
                    nc.tensor.matmul(
                        ptr[:], xt[j][:], AT[j][:], start=(j == 0), stop=(j == 3)
                    )
                if name == "AS":
                    nc.vector.tensor_mul(fs[:], C[:], ptr[:])
                else:
                    tmp = io.tile([BS, DIM], F32, tag="fs2")
                    nc.vector.tensor_mul(tmp[:], S[:], ptr[:])
                    nc.vector.tensor_sub(fs[:], fs[:], tmp[:])

            # ---- MLP ----
            def mlp_layer(l, in_tiles, in_tail, out_dim, bias, act_fn):
                """in_tiles: list of (xT_tile[128,64], W row offset); in_tail:
                ([2,64] tile, row offset) or None."""
                Wl = W_p[l]
                n_sizes = [512, 512, 512, 2] if out_dim == H else [512]
                psum = [
                    ps.tile([BS, n], F32, tag=f"ps{n_i}", name=f"ps{l}_{n_i}")
                    for n_i, n in enumerate(n_sizes)
                ]
                for xt, roff in in_tiles:
                    wt = wp.tile([128, out_dim], F32R, tag="wk")
                    half = out_dim // 2
                    nc.sync.dma_start(
                        out=wt[:, 0:half],
                        in_=Wl[roff : roff + 128, 0:half].bitcast(F32R),
                    )
                    nc.sync.dma_start(
                        out=wt[:, half:out_dim],
                        in_=Wl[roff : roff + 128, half:out_dim].bitcast(F32R),
                    )
                    off = 0
                    for n_i, n in enumerate(n_sizes):
                        nc.tensor.matmul(
                            psum[n_i][:],
                            xt[:],
                            wt[:, off : off + n],
                            start=(roff == in_tiles[0][1]),
                            stop=False,
                        )
                        off += n
                if in_tail is not None:
                    xt2, roff2 = in_tail
                    wt2 = wp.tile([2, out_dim], F32R, tag="wk2", bufs=2)
                    nc.sync.dma_start(out=wt2[:], in_=Wl[roff2 : roff2 + 2, :].bitcast(F32R))
                    off = 0
                    for n_i, n in enumerate(n_sizes):
                        nc.tensor.matmul(
                            psum[n_i][:], xt2[:], wt2[:, off : off + n],
                            start=False, stop=False,
                        )
                        off += n
                # bias via ones-row (K=1) matmul
                off = 0
                for n_i, n in enumerate(n_sizes):
                    nc.tensor.matmul(
                        psum[n_i][:], ones[:], bias[:, off : off + n],
                        start=False, stop=True,
                    )
                    off += n
                h = io.tile([BS, out_dim], F32, tag=f"h{l % 2}")
                off = 0
                for n_i, n in enumerate(n_sizes):
                    nc.scalar.activation(h[:, off : off + n], psum[n_i][:], act_fn)
                    off += n
                return h

            def transpose_h(h, l):
                tiles = []
                for j in range(12):
                    p = pst.tile([128, 64], F32, tag="pstT")
                    nc.tensor.transpose(p[:], h[:, j * 128 : (j + 1) * 128], id64[:])
                    ht = htp.tile([128, 64], F32R, tag=f"hT{j}")
                    nc.vector.tensor_copy(ht[:], p[:])
                    tiles.append((ht, j * 128))
                p2 = pst.tile([2, 64], F32, tag="pstA", bufs=1)
                nc.tensor.transpose(p2[:], h[:, 1536:1538], id64[:])
                ht2 = htp.tile([2, 64], F32R, tag="hTtail")
                nc.vector.tensor_copy(ht2[:], p2[:])
                return tiles, (ht2, 1536)

            l0_tiles = (
                [(xC[j], j * 128) for j in range(4)]
                + [(xS[j], 512 + j * 128) for j in range(4)]
                + [(xF[j], 1025 + j * 128) for j in range(4)]
            )
            h = mlp_layer(0, l0_tiles, None, H, b0p, AF.Tanh)
            for l in (1, 2):
                tiles, tail = transpose_h(h, l)
                h = mlp_layer(l, tiles, tail, H, brow[l], AF.Tanh)
            tiles, tail = transpose_h(h, 3)
            cforce = mlp_layer(3, tiles, tail, DIM, brow[3], AF.Copy)

            # ---- outputs ----
            out_sb = io.tile([BS, DIM + 1], F32, tag="osb")
            # force = cforce * fs / DIM + freqs
            fm = io.tile([BS, DIM], F32, tag="fm")
            nc.vector.tensor_mul(fm[:], cforce[:], fs[:])
            nc.vector.tensor_scalar_mul(fm[:], fm[:], 1.0 / DIM)
            nc.vector.tensor_add(out_sb[:, 0:DIM], fm[:], fr[:])
            # f1 = sum_i cforce^2
            sq = io.tile([BS, DIM], F32, tag="sq")
            nc.scalar.activation(
                sq[:], cforce[:], AF.Square, accum_out=out_sb[:, DIM : DIM + 1]
            )
            nc.sync.dma_start(out=out_p[:], in_=out_sb[:])


        for _rep in range(reps):
            _emit(_rep)

    _split_waits(nc)
    return nc


_NC_CACHE = {}


def kernel(**inputs):
    key = "nc"
    if key not in _NC_CACHE:
        _NC_CACHE[key] = _build()
    nc = _NC_CACHE[key]

    shared = {
        k: np.ascontiguousarray(inputs[k], dtype=np.float32)
        for k in ("t", "A", "W0", "b0", "W1", "b1", "W2", "b2", "W3", "b3")
    }
    y = np.asarray(inputs["y"], dtype=np.float32)
    freqs = np.asarray(inputs["freqs"], dtype=np.float32)
    in_maps = []
    for i in range(NCORES):
        m = dict(shared)
        m["y"] = np.ascontiguousarray(y[i * BS : (i + 1) * BS])
        m["freqs"] = np.ascontiguousarray(freqs[i * BS : (i + 1) * BS])
        in_maps.append(m)

    res = run_bass_kernel_spmd(nc, in_maps, core_ids=list(range(NCORES)))
    out = np.concatenate([res.results[i]["out"] for i in range(NCORES)], axis=0)
    return out.astype(np.float32)

